# revision 42
# baseline (speedup 1.0000x reference)
"""Self-contained Trainium2 Bass kernel: 16-head attention with RoPE (B=2, S=2048, D=2048).

Sharding: 8 cores = 2 (batch) x 4 (head groups of 4 heads / 512 cols).
Per core: QKV projections for its head group -> RoPE -> causal attention ->
per-(chunk, head) AllGather of attention outputs (X^T) within the 4-core
batch group -> column-sharded output projection. Host assembles by
concatenation only.

Pipeline (per q-chunk c of 512):
  proj V(c), Q(c)+RoPE, K(c)+RoPE, then per head: pipelined attention where
  scores run LAG k-tiles ahead of the PV matmuls so the PE queue never
  head-blocks on the exp chain; causal tri-mask adds and softmax rowsums
  (bf16 pair-adds + f32 chain) run on Vector; exp is the only Scalar work
  during attention. Attention outputs AllGather per head-pair (the CC core
  spends its first ~100us on runtime init, so chunk 0/1's outproj groups
  are deferred to attention(2)); outproj st-groups interleave into the next
  chunk's attention and the scheduler hoists their matmuls into exp-wait
  stalls. The last chunk runs as 384+128-wide q-pieces with per-head /
  single AllGathers so the true tail is one small gather + one st-group.
  A tiny warmup AllGather absorbs the CC cold-start under proj(0).

Dataflow is fully "transposed" so no on-chip transposes are needed:
  hiddenT [d, s] (host-pretransposed, bf16), streamed per chunk
  QT/KT   [dh, s] per head  (projection emits head-dim-major directly)
  S^T     [k, q] scores     (lhsT = KT tile, rhs = QT)
  P^T     [k, q] = exp(S^T + mask^T)   (no max subtraction; scores are O(1))
  colsums via all-ones [128,128] matmul -> sums arrive partition-broadcast
  O^T     [dh, q] = V^T @ P^T          (lhsT = V natural [s, dh])
  X^T     per-head AllGather on first axis
  out     [s, oc] (lhsT = X^T block, rhs = WoT)
RoPE de-interleave is folded into a host-side row permutation of Wq/Wk.
1/sqrt(DH) is folded into the Q rope tables.
"""

import math
from contextlib import ExitStack

import numpy as np
import ml_dtypes

B, S, D, H, DH = 2, 2048, 2048, 16, 128
NCORES = 8
GPC = 4            # cores per tensor-parallel group
HPC = H // GPC     # heads per core (4)
CW = HPC * DH      # 512 columns per core
NEG = -1e9
BF = ml_dtypes.bfloat16
QCH = 512          # q-chunk (moving free dim)
NQC = S // QCH     # 4
NDT = D // 128     # 16 d-tiles
NST = S // 128     # 16 s-tiles
LAG = 3            # PV matmul trails its score matmul by LAG k-tiles

REPLICA_GROUPS = [[0, 1, 2, 3], [4, 5, 6, 7]]

_built = {}


def _build(causal: bool, use_bias: bool):
    import concourse.bass as bass
    import concourse.tile as tile
    from concourse import bacc, mybir
    from concourse.tile_rust import add_dep_helper

    f32, bf16 = mybir.dt.float32, mybir.dt.bfloat16
    EXP = mybir.ActivationFunctionType.Exp
    IDN = mybir.ActivationFunctionType.Identity

    nc = bacc.Bacc("TRN2", target_bir_lowering=False, debug=False,
                   num_devices=NCORES)

    hT_d = nc.dram_tensor("hiddenT", [D, S], bf16, kind="ExternalInput")
    wq_d = nc.dram_tensor("wqT", [D, CW], bf16, kind="ExternalInput")
    wk_d = nc.dram_tensor("wkT", [D, CW], bf16, kind="ExternalInput")
    wv_d = nc.dram_tensor("wvT", [D, CW], bf16, kind="ExternalInput")
    wo_d = nc.dram_tensor("woT", [D, CW], bf16, kind="ExternalInput")
    cq_d = nc.dram_tensor("cq", [128, S], bf16, kind="ExternalInput")
    sq_d = nc.dram_tensor("sq", [128, S], bf16, kind="ExternalInput")
    ck_d = nc.dram_tensor("ck", [128, S], bf16, kind="ExternalInput")
    sk_d = nc.dram_tensor("sk", [128, S], bf16, kind="ExternalInput")
    if use_bias:
        bq_d = nc.dram_tensor("bqp", [128, HPC], f32, kind="ExternalInput")
        bk_d = nc.dram_tensor("bkp", [128, HPC], f32, kind="ExternalInput")
        bv_d = nc.dram_tensor("bv2", [1, CW], f32, kind="ExternalInput")
        bo_d = nc.dram_tensor("bo2", [1, CW], f32, kind="ExternalInput")
    if causal:
        dm_d = nc.dram_tensor("dmask", [128, 128], f32, kind="ExternalInput")
    else:
        mT_d = nc.dram_tensor("maskT", [S, S], bf16, kind="ExternalInput")
    out_d = nc.dram_tensor("out", [S, CW], f32, kind="ExternalOutput")

    with tile.TileContext(nc) as tc, ExitStack() as ctx:
        hp = ctx.enter_context(tc.tile_pool(name="hp", bufs=2 * NDT))
        xp = ctx.enter_context(tc.tile_pool(name="xp", bufs=NDT + 4))
        wp = ctx.enter_context(tc.tile_pool(name="wp", bufs=4 * NDT))
        qkp = ctx.enter_context(tc.tile_pool(name="qkp", bufs=2 * HPC))
        vp = ctx.enter_context(tc.tile_pool(name="vp", bufs=NST))
        cst = ctx.enter_context(tc.tile_pool(name="cst", bufs=1))
        ptp = ctx.enter_context(tc.tile_pool(name="ptp", bufs=7))
        accp = ctx.enter_context(tc.tile_pool(name="accp", bufs=2))
        rp = ctx.enter_context(tc.tile_pool(name="rp", bufs=2))
        op = ctx.enter_context(tc.tile_pool(name="op", bufs=3))
        ps_mm = ctx.enter_context(tc.tile_pool(name="ps_mm", bufs=2, space="PSUM"))
        ps_s = ctx.enter_context(tc.tile_pool(name="ps_s", bufs=3, space="PSUM"))
        ps_att = ctx.enter_context(tc.tile_pool(name="ps_att", bufs=3, space="PSUM"))
        dram = ctx.enter_context(tc.tile_pool(name="dram", bufs=1, space="DRAM"))

        # ---- first-needed data first: hT(chunk 0) + Wv interleaved ----
        wv_sb, wq_sb, wk_sb, wo_sb = [], [], [], []
        hTc0 = []
        for dt in range(NDT):
            w = wp.tile([128, CW], bf16, tag="w", name=f"wq{dt}")
            nc.sync.dma_start(w[:], wq_d[dt * 128:(dt + 1) * 128, :])
            wq_sb.append(w)
            t = hp.tile([128, QCH], bf16, tag="hT", name=f"hT0_{dt}")
            nc.sync.dma_start(t[:], hT_d[dt * 128:(dt + 1) * 128, 0:QCH])
            hTc0.append(t)
        # V weights next (used after Q)
        for dt in range(NDT):
            w = wp.tile([128, CW], bf16, tag="w", name=f"wv{dt}")
            nc.sync.dma_start(w[:], wv_d[dt * 128:(dt + 1) * 128, :])
            wv_sb.append(w)

        # ---- constants ----
        cq_sb = cst.tile([128, S], bf16, tag="cq", name="cq_sb")
        sq_sb = cst.tile([128, S], bf16, tag="sq", name="sq_sb")
        ck_sb = cst.tile([128, S], bf16, tag="ck", name="ck_sb")
        sk_sb = cst.tile([128, S], bf16, tag="sk", name="sk_sb")
        nc.sync.dma_start(cq_sb[:], cq_d[:])
        nc.sync.dma_start(sq_sb[:], sq_d[:])
        for dt in range(NDT):
            w = wp.tile([128, CW], bf16, tag="w", name=f"wk{dt}")
            nc.sync.dma_start(w[:], wk_d[dt * 128:(dt + 1) * 128, :])
            wk_sb.append(w)
        nc.sync.dma_start(ck_sb[:], ck_d[:])
        nc.sync.dma_start(sk_sb[:], sk_d[:])
        if use_bias:
            bq_sb = cst.tile([128, HPC], f32, tag="bq", name="bq_sb")
            bk_sb = cst.tile([128, HPC], f32, tag="bk", name="bk_sb")
            bv_sb = cst.tile([1, CW], f32, tag="bv", name="bv_sb")
            bo_sb = cst.tile([1, CW], f32, tag="bo", name="bo_sb")
            nc.sync.dma_start(bq_sb[:], bq_d[:])
            nc.sync.dma_start(bk_sb[:], bk_d[:])
            nc.sync.dma_start(bv_sb[:], bv_d[:])
            nc.sync.dma_start(bo_sb[:], bo_d[:])
            bvb_sb = cst.tile([128, CW], f32, tag="bvb", name="bvb_sb")
            bob_sb = cst.tile([128, CW], f32, tag="bob", name="bob_sb")
            nc.gpsimd.partition_broadcast(bvb_sb[:], bv_sb[0:1, :])
            nc.gpsimd.partition_broadcast(bob_sb[:], bo_sb[0:1, :])
        ones_sb = cst.tile([128, 128], bf16, tag="ones", name="ones_sb")
        nc.vector.memset(ones_sb[:], 1.0)
        # tiny warmup AllGather: pays the CC core's cold-start cost under
        # chunk 0's projections instead of before its first real gather
        wg_in = dram.tile([512, 64], bf16, tag="wg_in", name="wg_in")
        wg_out = dram.tile([GPC * 512, 64], bf16, tag="wg_out", name="wg_out")
        for r in range(4):
            nc.sync.dma_start(wg_in[r * 128:(r + 1) * 128, :],
                              ones_sb[:, 0:64])
        nc.gpsimd.collective_compute(
            "AllGather", mybir.AluOpType.bypass,
            replica_groups=REPLICA_GROUPS,
            ins=[wg_in[:].opt()], outs=[wg_out[:].opt()])
        if causal:
            tri_sb = cst.tile([128, 128], f32, tag="tri", name="tri_sb")
            nc.sync.dma_start(tri_sb[:], dm_d[:])

        # Wo streams in behind everything else
        for dt in range(NDT):
            t = wp.tile([128, CW], bf16, tag="w", name=f"wo{dt}")
            nc.sync.dma_start(t[:], wo_d[dt * 128:(dt + 1) * 128, :])
            wo_sb.append(t)

        # persistent KT (written chunk by chunk; all history needed) and V;
        # QT is per-chunk only
        ktr = [qkp.tile([128, S], bf16, tag="qk", name=f"ktr{m}", bufs=HPC)
               for m in range(HPC)]
        v_sb = [None] * NST

        def rope(dst, dsl, c_sb, s_sb, c, prefix, m):
            """RoPE dst[:, dsl] in place (4 DVE ops + 2 DMA swaps).
            rows 0:64 = "real"(a), 64:128 = "imag"(b):
              a' = a*cos - b*sin ; b' = b*cos + a*sin"""
            csl = slice(c * QCH, (c + 1) * QCH)
            t1 = rp.tile([128, QCH], bf16, tag="t1", name=f"{prefix}t1{m}_{c}")
            tc_ = rp.tile([128, QCH], bf16, tag="tc", name=f"{prefix}tc{m}_{c}")
            ts_ = rp.tile([128, QCH], bf16, tag="ts", name=f"{prefix}ts{m}_{c}")
            nc.sync.dma_start(t1[0:64, :], dst[64:128, dsl])
            nc.sync.dma_start(t1[64:128, :], dst[0:64, dsl])
            nc.vector.tensor_mul(tc_[:], dst[:, dsl], c_sb[:, csl])  # [a*c;b*c]
            nc.vector.tensor_mul(ts_[:], t1[:], s_sb[:, csl])        # [b*s;a*s]
            nc.vector.tensor_sub(dst[0:64, dsl], tc_[0:64, :], ts_[0:64, :])
            nc.vector.tensor_add(dst[64:128, dsl], tc_[64:128, :], ts_[64:128, :])

        def proj_head(c, hTc, w_sb, b_sb, c_sb, s_sb, dst, dsl, prefix, m):
            """One head's Q or K projection (16 accumulating MMs) + rope."""
            ps = ps_mm.tile([128, QCH], f32, tag="mm", name=f"{prefix}ps{m}_{c}")
            for dt in range(NDT):
                nc.tensor.matmul(ps[:], w_sb[dt][:, m * 128:(m + 1) * 128],
                                 hTc[dt][:], start=(dt == 0), stop=(dt == NDT - 1))
            if use_bias:
                nc.scalar.activation(dst[:, dsl], ps[:], IDN, bias=b_sb[:, m:m + 1])
            else:
                nc.scalar.activation(dst[:, dsl], ps[:], IDN)
            rope(dst, dsl, c_sb, s_sb, c, prefix, m)

        def proj_vq(c, hTcs):
            """Prefetch hT(c+1); project V (4 s-tiles) and all Q heads."""
            if c + 1 < NQC:
                nsl = slice((c + 1) * QCH, (c + 2) * QCH)
                nxt = []
                for dt in range(NDT):
                    t = hp.tile([128, QCH], bf16, tag="hT",
                                name=f"hT{c + 1}_{dt}")
                    nc.sync.dma_start(t[:], hT_d[dt * 128:(dt + 1) * 128, nsl])
                    nxt.append(t)
                hTcs[c + 1] = nxt
            hTc = hTcs[c]
            # Q first: roped Q plus old K/V is all the scheduler needs to
            # hoist attention's old-tile scores (and their exps) into the
            # V/K projection phase
            qtrc = [qkp.tile([128, QCH], bf16, tag="qtc", name=f"qtc{c}_{m}",
                             bufs=(HPC + 1 if causal else 4 * HPC))
                    for m in range(HPC)]
            for m in range(HPC):
                proj_head(c, hTc, wq_sb, bq_sb if use_bias else None,
                          cq_sb, sq_sb, qtrc[m], slice(0, QCH), "q", m)
            for sti in range(4):
                st = 4 * c + sti
                ps = ps_mm.tile([128, CW], f32, tag="mm", name=f"psv{st}")
                for dt in range(NDT):
                    nc.tensor.matmul(ps[:], hTc[dt][:, sti * 128:(sti + 1) * 128],
                                     wv_sb[dt][:],
                                     start=(dt == 0), stop=(dt == NDT - 1))
                vt = vp.tile([128, CW], bf16, tag="v", name=f"v{st}")
                if use_bias:
                    nc.vector.tensor_add(vt[:], ps[:], bvb_sb[:])
                else:
                    nc.vector.tensor_copy(vt[:], ps[:])
                v_sb[st] = vt
            return hTc, qtrc

        # per-(chunk, head-pair) AllGather buffers
        agout = {}     # (key, group) -> DRAM tile [GPC*gs*128, qw]
        agin_pend = {}
        AG_GS = {"La": 1, "Lb": 1}

        xt_tiles = {}

        def load_xt(key, qw=QCH):
            """Fetch the gathered X^T tiles for AG key (one contiguous DMA
            per (head, rank) block); shared by the matching outproj groups."""
            for hh in range(HPC):
                for g in range(GPC):
                    t = xp.tile([128, QCH], bf16, tag="xt",
                                name=f"xt{key}_{hh}_{g}")
                    gs = AG_GS.get(key, 2)
                    row = g * gs * 128 + (hh % gs) * 128
                    nc.sync.dma_start(
                        t[:, 0:qw], agout[(key, hh // gs)][row:row + 128, :])
                    xt_tiles[(key, hh, g)] = t

        def outproj_group(qc, st, after_mm=None, xt_key=None, stoff=None,
                          of_vec=False):
            """out rows [qc*QCH + st*128 : +128] = X^T chunk-slice @ WoT.
            dt accumulation ordered h-major so late AGs are needed last."""
            if xt_key is None:
                xt_key = qc
            if stoff is None:
                stoff = st * 128
            stsl = slice(stoff, stoff + 128)
            ps = ps_mm.tile([128, CW], f32, tag="mm", name=f"pso{qc}_{st}")
            n = 0
            for hh in range(HPC):
                for g in range(GPC):
                    dt = 4 * g + hh
                    mm = nc.tensor.matmul(ps[:], xt_tiles[(xt_key, hh, g)][:, stsl],
                                          wo_sb[dt][:],
                                          start=(n == 0), stop=(n == NDT - 1))
                    if n == 0 and after_mm is not None:
                        # keep outproj behind the current attention head in PE
                        # program order; the static scheduler would hoist it
                        # ahead of the gather outputs otherwise
                        add_dep_helper(mm.ins, after_mm.ins, sync=False,
                                       reason="outproj after attn head")
                    n += 1
            of = op.tile([128, CW], f32, tag="of", name=f"of{qc}_{st}", bufs=2)
            if use_bias:
                nc.vector.tensor_add(of[:], ps[:], bob_sb[:])
            elif of_vec:
                nc.vector.tensor_copy(of[:], ps[:])
            else:
                nc.scalar.activation(of[:], ps[:], IDN)
            row = qc * QCH + st * 128
            nc.sync.dma_start(out_d[row:row + 128, :], of[:])

        def attn_head(c, h, qtrc, nk, qoff=0, qw=QCH, key=None):
            """Pipelined attention for (chunk c, head h) over q columns
            [qoff, qoff+qw) of the chunk: scores run LAG k-tiles ahead of the
            PV matmuls; rowsums via bf16 pairs + f32 chain on vector; fires
            AG(key) at the end. Returns the rowsum matmul for ordering."""
            if key is None:
                key = c
            base = c * QCH + qoff      # global q start of this sub-range
            pv = ps_att.tile([128, QCH], f32, tag="att", name=f"pv{h}_{key}")
            pts = {}
            c0s = {}
            sacc = None      # running f32 sum of pair tiles
            pair_hold = None
            quad = None

            def emit_pv(j):
                c0 = c0s[j]
                nc.tensor.matmul(
                    pv[:, c0:qw], v_sb[j][:, h * 128:(h + 1) * 128],
                    pts[j][:, c0:qw], start=(j == 0), stop=(j == nk - 1))

            for ki in range(nk):
                rel = 128 * ki - base if causal else -1
                c0 = max(0, rel)
                c0s[ki] = c0
                ss = ps_s.tile([128, QCH], f32, tag="s",
                               name=f"ss{h}_{key}_{ki}")
                nc.tensor.matmul(
                    ss[:, c0:qw], ktr[h][:, ki * 128:(ki + 1) * 128],
                    qtrc[h][:, qoff + c0:qoff + qw], start=True, stop=True)
                if causal:
                    if rel >= 0:
                        nc.vector.tensor_add(ss[:, c0:c0 + 128],
                                             ss[:, c0:c0 + 128], tri_sb[:])
                else:
                    mt = ptp.tile([128, QCH], bf16, tag="mt",
                                  name=f"mt{h}_{key}_{ki}", bufs=4)
                    nc.sync.dma_start(
                        mt[:], mT_d[ki * 128:(ki + 1) * 128,
                                    c * QCH:(c + 1) * QCH])
                    nc.vector.tensor_add(ss[:], ss[:], mt[:])
                pt = ptp.tile([128, QCH], bf16, tag="pt",
                              name=f"pt{h}_{key}_{ki}")
                nc.scalar.activation(pt[:, c0:qw], ss[:, c0:qw], EXP)
                pts[ki] = pt
                # ---- rowsum accumulation (vector) ----
                if causal and rel >= 0:
                    # diagonal tiles fold into one bf16 tile
                    if c0 == 0:
                        quad = accp.tile([128, QCH], bf16, tag="quad",
                                         name=f"qd{h}_{key}")
                        nc.vector.tensor_copy(quad[:, 0:qw], pt[:, 0:qw])
                    else:
                        nc.vector.tensor_add(quad[:, c0:qw], quad[:, c0:qw],
                                             pt[:, c0:qw])
                else:
                    # full tiles pair up in bf16, then a f32 chain
                    if pair_hold is None:
                        pair_hold = pt
                    else:
                        pair = accp.tile([128, QCH], bf16, tag="pair",
                                         name=f"pr{h}_{key}_{ki}")
                        nc.vector.tensor_add(pair[:, 0:qw], pair_hold[:, 0:qw],
                                             pt[:, 0:qw])
                        pair_hold = None
                        if sacc is None:
                            sacc = accp.tile([128, QCH], f32, tag="sacc",
                                             name=f"sa{h}_{key}")
                            nc.vector.tensor_copy(sacc[:, 0:qw], pair[:, 0:qw])
                        else:
                            nc.vector.tensor_add(sacc[:, 0:qw], sacc[:, 0:qw],
                                                 pair[:, 0:qw])
                if ki >= LAG:
                    emit_pv(ki - LAG)
            for j in range(max(0, nk - LAG), nk):
                emit_pv(j)
            # odd leftover full tile (possible for sub-ranges)
            if pair_hold is not None:
                if sacc is None:
                    sacc = accp.tile([128, QCH], f32, tag="sacc",
                                     name=f"sa{h}_{key}")
                    nc.vector.tensor_copy(sacc[:, 0:qw], pair_hold[:, 0:qw])
                else:
                    nc.vector.tensor_add(sacc[:, 0:qw], sacc[:, 0:qw],
                                         pair_hold[:, 0:qw])
            # merge rowsums -> bf16 saccb for the ones-matmul reduce
            if causal and sacc is None:
                saccb = quad
            else:
                saccb = accp.tile([128, QCH], bf16, tag="saccb",
                                  name=f"sb{h}_{key}")
                if causal:
                    nc.vector.tensor_add(saccb[:, 0:qw], sacc[:, 0:qw],
                                         quad[:, 0:qw])
                else:
                    nc.vector.tensor_copy(saccb[:, 0:qw], sacc[:, 0:qw])
            # partition-reduce+broadcast the rowsums in one bf16 matmul
            sm = ps_att.tile([128, QCH], f32, tag="att", name=f"sm{h}_{key}")
            sm_mm = nc.tensor.matmul(sm[:, 0:qw], ones_sb[:], saccb[:, 0:qw],
                                     start=True, stop=True)
            recb = op.tile([128, QCH], f32, tag="recb",
                           name=f"recb{h}_{key}", bufs=2)
            nc.vector.reciprocal_approx_fast(out=recb[:, 0:qw], in_=sm[:, 0:qw])
            ot = op.tile([128, QCH], bf16, tag="ot", name=f"ot{h}_{key}",
                         bufs=3)
            nc.vector.tensor_mul(ot[:, 0:qw], pv[:, 0:qw], recb[:, 0:qw])
            gs = AG_GS.get(key, 2)
            pr, sl = divmod(h, gs)
            if sl == 0:
                agin = dram.tile([gs * 128, qw], bf16, tag=f"agin{key}_{pr}",
                                 name=f"agin{key}_{pr}")
                agin_pend[(key, pr)] = agin
            else:
                agin = agin_pend[(key, pr)]
            nc.sync.dma_start(agin[sl * 128:sl * 128 + 128, :], ot[:, 0:qw])
            if sl == gs - 1:
                ago = dram.tile([GPC * gs * 128, qw], bf16,
                                tag=f"agout{key}_{pr}",
                                name=f"agout{key}_{pr}")
                nc.gpsimd.collective_compute(
                    "AllGather", mybir.AluOpType.bypass,
                    replica_groups=REPLICA_GROUPS,
                    ins=[agin[:].opt()], outs=[ago[:].opt()])
                agout[(key, pr)] = ago
            return sm_mm

        # ---- main pipeline over q-chunks ----
        # Per block: V proj, Q proj, then per head: K proj for that head,
        # attention(c, h), outproj(c-1) st-groups. Interleaving K per head
        # keeps each engine's FIFO aligned with the intended overlap (exps
        # are never queued behind later proj copies).
        # Chunk 0's four AGs serialize on the CC core right after the tiny
        # attention(0), so its outproj groups wait until attention(1)'s later
        # heads to avoid stalling on the gather outputs.
        hTcs = {0: hTc0}
        if causal:
            for c in range(NQC):
                hTc, qtrc = proj_vq(c, hTcs)
                for h in range(HPC):
                    proj_head(c, hTc, wk_sb, bk_sb if use_bias else None,
                              ck_sb, sk_sb, ktr[h],
                              slice(c * QCH, (c + 1) * QCH), "k", h)
                # the CC core spends its first ~100us on runtime init, so
                # chunk 0/1's gathers land late; both chunks' outproj groups
                # run interleaved into attention(2) instead
                if c == 2:
                    load_xt(0)
                elif c == 3:
                    load_xt(2)
                sched = {1: [[], [], [], []],
                         2: [[(0, 0), (0, 1)], [(0, 2), (0, 3)],
                             [(1, 0), (1, 1)], [(1, 2), (1, 3)]]}.get(
                    c, [[(c - 1, 0)], [(c - 1, 1)], [(c - 1, 2)],
                        [(c - 1, 3)]])
                if c < NQC - 1:
                    for h in range(HPC):
                        attn_head(c, h, qtrc, 4 * c + 4)
                        if c > 0:
                            for qs, st in sched[h]:
                                outproj_group(qs, st, None)
                        if c == 2 and h == 1:
                            # xt(0) buffers free once outproj(0) is done
                            load_xt(1)
                else:
                    # last chunk: a 384-wide piece then a 128-wide piece, so
                    # most of the final outproj overlaps the second piece and
                    # the true tail is one small AllGather + one st-group
                    WA, WB = 384, 128
                    for h in range(HPC):
                        attn_head(c, h, qtrc, 4 * c + 3, qoff=0,
                                  qw=WA, key="La")
                        outproj_group(c - 1, h, None)
                    for h in range(HPC):
                        attn_head(c, h, qtrc, 4 * c + 4, qoff=WA,
                                  qw=WB, key="Lb")
                    load_xt("La", WA)
                    for st in range(3):
                        outproj_group(c, st, None, xt_key="La",
                                      stoff=st * 128, of_vec=True)
                    load_xt("Lb", WB)
                    outproj_group(c, 3, None, xt_key="Lb", stoff=0,
                                  of_vec=True)
        else:
            qtrcs = []
            for c in range(NQC):
                hTc, qtrc = proj_vq(c, hTcs)
                for h in range(HPC):
                    proj_head(c, hTc, wk_sb, bk_sb if use_bias else None,
                              ck_sb, sk_sb, ktr[h],
                              slice(c * QCH, (c + 1) * QCH), "k", h)
                qtrcs.append(qtrc)
            for c in range(NQC):
                if c > 0:
                    load_xt(c - 1)
                for h in range(HPC):
                    sm_mm = attn_head(c, h, qtrcs[c], NST)
                    if c > 0:
                        outproj_group(c - 1, h, sm_mm)
            load_xt(NQC - 1)
            for st in range(4):
                outproj_group(NQC - 1, st)

    nc.compile()
    return nc


def _get_built(causal: bool, use_bias: bool):
    key = (causal, use_bias)
    if key not in _built:
        _built[key] = _build(causal, use_bias)
    return _built[key]


def _prep_inputs(inputs, causal, use_bias):
    hs = np.asarray(inputs["hidden_states"], np.float32)
    fc = np.asarray(inputs["freqs_cis"], np.float32)
    Wq = np.asarray(inputs["Wq"], np.float32)
    Wk = np.asarray(inputs["Wk"], np.float32)
    Wv = np.asarray(inputs["Wv"], np.float32)
    Wo = np.asarray(inputs["Wo"], np.float32)
    bq = np.asarray(inputs["bq"], np.float32)
    bk = np.asarray(inputs["bk"], np.float32)
    bv = np.asarray(inputs["bv"], np.float32)
    bo = np.asarray(inputs["bo"], np.float32)

    # de-interleave permutation per 128-row head block: [0,2,..,126, 1,3,..,127]
    perm1 = np.concatenate([np.arange(0, DH, 2), np.arange(1, DH, 2)])
    permC = (np.arange(CW) // DH) * DH  # head base offsets
    perm = permC + perm1[np.arange(CW) % DH]

    scale = 1.0 / math.sqrt(DH)
    cos = np.concatenate([fc[:, :, 0].T, fc[:, :, 0].T])  # [128, S], dup halves
    sin = np.concatenate([fc[:, :, 1].T, fc[:, :, 1].T])
    cq = np.ascontiguousarray(cos * scale).astype(BF)
    sq = np.ascontiguousarray(sin * scale).astype(BF)
    ck = np.ascontiguousarray(cos).astype(BF)
    sk = np.ascontiguousarray(sin).astype(BF)

    if causal:
        tri = np.where(np.arange(128)[:, None] > np.arange(128)[None, :],
                       np.float32(NEG), np.float32(0.0)).astype(np.float32)
    else:
        maskT = np.ascontiguousarray(
            np.asarray(inputs["mask"], np.float32)[0, 0].T).astype(BF)

    hTb = [np.ascontiguousarray(hs[b].T).astype(BF) for b in range(B)]

    in_maps = []
    for c in range(NCORES):
        b, hg = divmod(c, GPC)
        sl = slice(CW * hg, CW * (hg + 1))
        wq_s = Wq[sl][perm]
        wk_s = Wk[sl][perm]
        m = {
            "hiddenT": hTb[b],
            "wqT": np.ascontiguousarray(wq_s.T).astype(BF),
            "wkT": np.ascontiguousarray(wk_s.T).astype(BF),
            "wvT": np.ascontiguousarray(Wv[sl].T).astype(BF),
            "woT": np.ascontiguousarray(Wo[sl].T).astype(BF),
            "cq": cq, "sq": sq, "ck": ck, "sk": sk,
        }
        if use_bias:
            m["bqp"] = np.ascontiguousarray(
                bq[sl][perm].reshape(HPC, 128).T).astype(np.float32)
            m["bkp"] = np.ascontiguousarray(
                bk[sl][perm].reshape(HPC, 128).T).astype(np.float32)
            m["bv2"] = bv[sl].reshape(1, CW).astype(np.float32)
            m["bo2"] = bo[sl].reshape(1, CW).astype(np.float32)
        if causal:
            m["dmask"] = tri
        else:
            m["maskT"] = maskT
        in_maps.append(m)
    return in_maps


def _is_causal(mask):
    mask = np.asarray(mask, np.float32)
    if mask.shape != (1, 1, S, S):
        return False
    m = mask[0, 0]
    expect = np.triu(np.full((S, S), np.float32(NEG)), k=1)
    return bool(np.array_equal(m, expect))


def run_on_cores(inputs, trace=False):
    """Compile+run; returns BassKernelResults."""
    from concourse.bass_utils import run_bass_kernel_spmd
    causal = _is_causal(inputs["mask"])
    use_bias = any(
        np.any(np.asarray(inputs[k])) for k in ("bq", "bk", "bv", "bo"))
    nc = _get_built(causal, use_bias)
    in_maps = _prep_inputs(inputs, causal, use_bias)
    r = run_bass_kernel_spmd(nc, in_maps, list(range(NCORES)), trace=trace)
    return r


def kernel(**inputs) -> np.ndarray:
    r = run_on_cores(inputs)
    out = np.empty((B, S, D), np.float32)
    for c in range(NCORES):
        b, hg = divmod(c, GPC)
        out[b, :, CW * hg:CW * (hg + 1)] = r.results[c]["out"]
    return out


# revision 43
# speedup vs baseline: 1.0685x; 1.0685x over previous
"""Self-contained Trainium2 Bass kernel: 16-head attention with RoPE (B=2, S=2048, D=2048).

Sharding: 8 cores = 2 (batch) x 4 (head groups of 4 heads / 512 cols).
Per core: QKV projections for its head group -> RoPE -> causal attention ->
per-(chunk, head) AllGather of attention outputs (X^T) within the 4-core
batch group -> column-sharded output projection. Host assembles by
concatenation only.

Pipeline (per q-chunk c of 512):
  proj V(c), Q(c)+RoPE, K(c)+RoPE, then per head: pipelined attention where
  scores run LAG k-tiles ahead of the PV matmuls so the PE queue never
  head-blocks on the exp chain; causal tri-mask adds and softmax rowsums
  (bf16 pair-adds + f32 chain) run on Vector; exp is the only Scalar work
  during attention. Attention outputs AllGather per head-pair (the CC core
  spends its first ~100us on runtime init, so chunk 0/1's outproj groups
  are deferred to attention(2)); outproj st-groups interleave into the next
  chunk's attention and the scheduler hoists their matmuls into exp-wait
  stalls. The last chunk runs as 384+128-wide q-pieces with per-head /
  single AllGathers so the true tail is one small gather + one st-group.
  A tiny warmup AllGather absorbs the CC cold-start under proj(0).

Dataflow is fully "transposed" so no on-chip transposes are needed:
  hiddenT [d, s] (host-pretransposed, bf16), streamed per chunk
  QT/KT   [dh, s] per head  (projection emits head-dim-major directly)
  S^T     [k, q] scores     (lhsT = KT tile, rhs = QT)
  P^T     [k, q] = exp(S^T + mask^T)   (no max subtraction; scores are O(1))
  colsums via all-ones [128,128] matmul -> sums arrive partition-broadcast
  O^T     [dh, q] = V^T @ P^T          (lhsT = V natural [s, dh])
  X^T     per-head AllGather on first axis
  out     [s, oc] (lhsT = X^T block, rhs = WoT)
RoPE de-interleave is folded into a host-side row permutation of Wq/Wk.
1/sqrt(DH) is folded into the Q rope tables.
"""

import math
from contextlib import ExitStack

import numpy as np
import ml_dtypes

B, S, D, H, DH = 2, 2048, 2048, 16, 128
NCORES = 8
GPC = 4            # cores per tensor-parallel group
HPC = H // GPC     # heads per core (4)
CW = HPC * DH      # 512 columns per core
NEG = -1e9
BF = ml_dtypes.bfloat16
QCH = 512          # q-chunk (moving free dim)
NQC = S // QCH     # 4
NDT = D // 128     # 16 d-tiles
NST = S // 128     # 16 s-tiles
LAG = 3            # PV matmul trails its score matmul by LAG k-tiles

REPLICA_GROUPS = [[0, 1, 2, 3], [4, 5, 6, 7]]

_built = {}


def _build(causal: bool, use_bias: bool):
    import concourse.bass as bass
    import concourse.tile as tile
    from concourse import bacc, mybir
    from concourse.tile_rust import add_dep_helper

    f32, bf16 = mybir.dt.float32, mybir.dt.bfloat16
    EXP = mybir.ActivationFunctionType.Exp
    IDN = mybir.ActivationFunctionType.Identity

    nc = bacc.Bacc("TRN2", target_bir_lowering=False, debug=False,
                   num_devices=NCORES)

    hT_d = nc.dram_tensor("hiddenT", [D, S], bf16, kind="ExternalInput")
    wq_d = nc.dram_tensor("wqT", [D, CW], bf16, kind="ExternalInput")
    wk_d = nc.dram_tensor("wkT", [D, CW], bf16, kind="ExternalInput")
    wv_d = nc.dram_tensor("wvT", [D, CW], bf16, kind="ExternalInput")
    wo_d = nc.dram_tensor("woT", [D, CW], bf16, kind="ExternalInput")
    cq_d = nc.dram_tensor("cq", [128, S], bf16, kind="ExternalInput")
    sq_d = nc.dram_tensor("sq", [128, S], bf16, kind="ExternalInput")
    ck_d = nc.dram_tensor("ck", [128, S], bf16, kind="ExternalInput")
    sk_d = nc.dram_tensor("sk", [128, S], bf16, kind="ExternalInput")
    if use_bias:
        bq_d = nc.dram_tensor("bqp", [128, HPC], f32, kind="ExternalInput")
        bk_d = nc.dram_tensor("bkp", [128, HPC], f32, kind="ExternalInput")
        bv_d = nc.dram_tensor("bv2", [1, CW], f32, kind="ExternalInput")
        bo_d = nc.dram_tensor("bo2", [1, CW], f32, kind="ExternalInput")
    if causal:
        dm_d = nc.dram_tensor("dmask", [128, 128], f32, kind="ExternalInput")
    else:
        mT_d = nc.dram_tensor("maskT", [S, S], bf16, kind="ExternalInput")
    out_d = nc.dram_tensor("out", [S, CW], f32, kind="ExternalOutput")

    with tile.TileContext(nc) as tc, ExitStack() as ctx:
        hp = ctx.enter_context(tc.tile_pool(name="hp", bufs=2 * NDT))
        xp = ctx.enter_context(tc.tile_pool(name="xp", bufs=NDT + 4))
        wp = ctx.enter_context(tc.tile_pool(name="wp", bufs=4 * NDT))
        qkp = ctx.enter_context(tc.tile_pool(name="qkp", bufs=2 * HPC))
        vp = ctx.enter_context(tc.tile_pool(name="vp", bufs=NST))
        cst = ctx.enter_context(tc.tile_pool(name="cst", bufs=1))
        ptp = ctx.enter_context(tc.tile_pool(name="ptp", bufs=7))
        accp = ctx.enter_context(tc.tile_pool(name="accp", bufs=2))
        rp = ctx.enter_context(tc.tile_pool(name="rp", bufs=2))
        op = ctx.enter_context(tc.tile_pool(name="op", bufs=3))
        ps_mm = ctx.enter_context(tc.tile_pool(name="ps_mm", bufs=2, space="PSUM"))
        ps_s = ctx.enter_context(tc.tile_pool(name="ps_s", bufs=3, space="PSUM"))
        ps_att = ctx.enter_context(tc.tile_pool(name="ps_att", bufs=3, space="PSUM"))
        dram = ctx.enter_context(tc.tile_pool(name="dram", bufs=1, space="DRAM"))

        # ---- first-needed data first: hT(chunk 0) + Wv interleaved ----
        wv_sb, wq_sb, wk_sb, wo_sb = [], [], [], []
        hTc0 = []
        for dt in range(NDT):
            w = wp.tile([128, CW], bf16, tag="w", name=f"wv{dt}")
            nc.sync.dma_start(w[:], wv_d[dt * 128:(dt + 1) * 128, :])
            wv_sb.append(w)
            t = hp.tile([128, QCH], bf16, tag="hT", name=f"hT0_{dt}")
            nc.sync.dma_start(t[:], hT_d[dt * 128:(dt + 1) * 128, 0:QCH])
            hTc0.append(t)
        # Q weights next (used before K)
        for dt in range(NDT):
            w = wp.tile([128, CW], bf16, tag="w", name=f"wq{dt}")
            nc.sync.dma_start(w[:], wq_d[dt * 128:(dt + 1) * 128, :])
            wq_sb.append(w)

        # ---- constants ----
        cq_sb = cst.tile([128, S], bf16, tag="cq", name="cq_sb")
        sq_sb = cst.tile([128, S], bf16, tag="sq", name="sq_sb")
        ck_sb = cst.tile([128, S], bf16, tag="ck", name="ck_sb")
        sk_sb = cst.tile([128, S], bf16, tag="sk", name="sk_sb")
        nc.sync.dma_start(cq_sb[:], cq_d[:])
        nc.sync.dma_start(sq_sb[:], sq_d[:])
        for dt in range(NDT):
            w = wp.tile([128, CW], bf16, tag="w", name=f"wk{dt}")
            nc.sync.dma_start(w[:], wk_d[dt * 128:(dt + 1) * 128, :])
            wk_sb.append(w)
        nc.sync.dma_start(ck_sb[:], ck_d[:])
        nc.sync.dma_start(sk_sb[:], sk_d[:])
        if use_bias:
            bq_sb = cst.tile([128, HPC], f32, tag="bq", name="bq_sb")
            bk_sb = cst.tile([128, HPC], f32, tag="bk", name="bk_sb")
            bv_sb = cst.tile([1, CW], f32, tag="bv", name="bv_sb")
            bo_sb = cst.tile([1, CW], f32, tag="bo", name="bo_sb")
            nc.sync.dma_start(bq_sb[:], bq_d[:])
            nc.sync.dma_start(bk_sb[:], bk_d[:])
            nc.sync.dma_start(bv_sb[:], bv_d[:])
            nc.sync.dma_start(bo_sb[:], bo_d[:])
            bvb_sb = cst.tile([128, CW], f32, tag="bvb", name="bvb_sb")
            bob_sb = cst.tile([128, CW], f32, tag="bob", name="bob_sb")
            nc.gpsimd.partition_broadcast(bvb_sb[:], bv_sb[0:1, :])
            nc.gpsimd.partition_broadcast(bob_sb[:], bo_sb[0:1, :])
        ones_sb = cst.tile([128, 128], bf16, tag="ones", name="ones_sb")
        nc.vector.memset(ones_sb[:], 1.0)
        # tiny warmup AllGather: pays the CC core's cold-start cost under
        # chunk 0's projections instead of before its first real gather
        wg_in = dram.tile([512, 64], bf16, tag="wg_in", name="wg_in")
        wg_out = dram.tile([GPC * 512, 64], bf16, tag="wg_out", name="wg_out")
        for r in range(4):
            nc.sync.dma_start(wg_in[r * 128:(r + 1) * 128, :],
                              ones_sb[:, 0:64])
        nc.gpsimd.collective_compute(
            "AllGather", mybir.AluOpType.bypass,
            replica_groups=REPLICA_GROUPS,
            ins=[wg_in[:].opt()], outs=[wg_out[:].opt()])
        if causal:
            tri_sb = cst.tile([128, 128], f32, tag="tri", name="tri_sb")
            nc.sync.dma_start(tri_sb[:], dm_d[:])

        # Wo streams in behind everything else
        for dt in range(NDT):
            t = wp.tile([128, CW], bf16, tag="w", name=f"wo{dt}")
            nc.sync.dma_start(t[:], wo_d[dt * 128:(dt + 1) * 128, :])
            wo_sb.append(t)

        # persistent KT (written chunk by chunk; all history needed) and V;
        # QT is per-chunk only
        ktr = [qkp.tile([128, S], bf16, tag="qk", name=f"ktr{m}", bufs=HPC)
               for m in range(HPC)]
        v_sb = [None] * NST

        def rope(dst, dsl, c_sb, s_sb, c, prefix, m):
            """RoPE dst[:, dsl] in place (4 DVE ops + 2 DMA swaps).
            rows 0:64 = "real"(a), 64:128 = "imag"(b):
              a' = a*cos - b*sin ; b' = b*cos + a*sin"""
            csl = slice(c * QCH, (c + 1) * QCH)
            t1 = rp.tile([128, QCH], bf16, tag="t1", name=f"{prefix}t1{m}_{c}")
            tc_ = rp.tile([128, QCH], bf16, tag="tc", name=f"{prefix}tc{m}_{c}")
            ts_ = rp.tile([128, QCH], bf16, tag="ts", name=f"{prefix}ts{m}_{c}")
            nc.sync.dma_start(t1[0:64, :], dst[64:128, dsl])
            nc.sync.dma_start(t1[64:128, :], dst[0:64, dsl])
            nc.vector.tensor_mul(tc_[:], dst[:, dsl], c_sb[:, csl])  # [a*c;b*c]
            nc.vector.tensor_mul(ts_[:], t1[:], s_sb[:, csl])        # [b*s;a*s]
            nc.vector.tensor_sub(dst[0:64, dsl], tc_[0:64, :], ts_[0:64, :])
            nc.vector.tensor_add(dst[64:128, dsl], tc_[64:128, :], ts_[64:128, :])

        def proj_head(c, hTc, w_sb, b_sb, c_sb, s_sb, dst, dsl, prefix, m):
            """One head's Q or K projection (16 accumulating MMs) + rope."""
            ps = ps_mm.tile([128, QCH], f32, tag="mm", name=f"{prefix}ps{m}_{c}")
            for dt in range(NDT):
                nc.tensor.matmul(ps[:], w_sb[dt][:, m * 128:(m + 1) * 128],
                                 hTc[dt][:], start=(dt == 0), stop=(dt == NDT - 1))
            if use_bias:
                nc.scalar.activation(dst[:, dsl], ps[:], IDN, bias=b_sb[:, m:m + 1])
            else:
                nc.scalar.activation(dst[:, dsl], ps[:], IDN)
            rope(dst, dsl, c_sb, s_sb, c, prefix, m)

        def proj_vq(c, hTcs):
            """Prefetch hT(c+1); project V (4 s-tiles) and all Q heads."""
            if c + 1 < NQC:
                nsl = slice((c + 1) * QCH, (c + 2) * QCH)
                nxt = []
                for dt in range(NDT):
                    t = hp.tile([128, QCH], bf16, tag="hT",
                                name=f"hT{c + 1}_{dt}")
                    nc.sync.dma_start(t[:], hT_d[dt * 128:(dt + 1) * 128, nsl])
                    nxt.append(t)
                hTcs[c + 1] = nxt
            hTc = hTcs[c]
            for sti in range(4):
                st = 4 * c + sti
                ps = ps_mm.tile([128, CW], f32, tag="mm", name=f"psv{st}")
                for dt in range(NDT):
                    nc.tensor.matmul(ps[:], hTc[dt][:, sti * 128:(sti + 1) * 128],
                                     wv_sb[dt][:],
                                     start=(dt == 0), stop=(dt == NDT - 1))
                vt = vp.tile([128, CW], bf16, tag="v", name=f"v{st}")
                if use_bias:
                    nc.vector.tensor_add(vt[:], ps[:], bvb_sb[:])
                else:
                    nc.vector.tensor_copy(vt[:], ps[:])
                v_sb[st] = vt
            qtrc = [qkp.tile([128, QCH], bf16, tag="qtc", name=f"qtc{c}_{m}",
                             bufs=(HPC + 1 if causal else 4 * HPC))
                    for m in range(HPC)]
            for m in range(HPC):
                proj_head(c, hTc, wq_sb, bq_sb if use_bias else None,
                          cq_sb, sq_sb, qtrc[m], slice(0, QCH), "q", m)
            return hTc, qtrc

        # per-(chunk, head-pair) AllGather buffers
        agout = {}     # (key, group) -> DRAM tile [GPC*gs*128, qw]
        agin_pend = {}
        AG_GS = {"La": 1, "Lb": 1}

        xt_tiles = {}

        def load_xt(key, qw=QCH):
            """Fetch the gathered X^T tiles for AG key (one contiguous DMA
            per (head, rank) block); shared by the matching outproj groups."""
            for hh in range(HPC):
                for g in range(GPC):
                    t = xp.tile([128, QCH], bf16, tag="xt",
                                name=f"xt{key}_{hh}_{g}")
                    gs = AG_GS.get(key, 2)
                    row = g * gs * 128 + (hh % gs) * 128
                    nc.sync.dma_start(
                        t[:, 0:qw], agout[(key, hh // gs)][row:row + 128, :])
                    xt_tiles[(key, hh, g)] = t

        def outproj_group(qc, st, after_mm=None, xt_key=None, stoff=None,
                          of_vec=False):
            """out rows [qc*QCH + st*128 : +128] = X^T chunk-slice @ WoT.
            dt accumulation ordered h-major so late AGs are needed last."""
            if xt_key is None:
                xt_key = qc
            if stoff is None:
                stoff = st * 128
            stsl = slice(stoff, stoff + 128)
            ps = ps_mm.tile([128, CW], f32, tag="mm", name=f"pso{qc}_{st}")
            n = 0
            for hh in range(HPC):
                for g in range(GPC):
                    dt = 4 * g + hh
                    mm = nc.tensor.matmul(ps[:], xt_tiles[(xt_key, hh, g)][:, stsl],
                                          wo_sb[dt][:],
                                          start=(n == 0), stop=(n == NDT - 1))
                    if n == 0 and after_mm is not None:
                        # keep outproj behind the current attention head in PE
                        # program order; the static scheduler would hoist it
                        # ahead of the gather outputs otherwise
                        add_dep_helper(mm.ins, after_mm.ins, sync=False,
                                       reason="outproj after attn head")
                    n += 1
            of = op.tile([128, CW], f32, tag="of", name=f"of{qc}_{st}", bufs=2)
            if use_bias:
                nc.vector.tensor_add(of[:], ps[:], bob_sb[:])
            elif of_vec:
                nc.vector.tensor_copy(of[:], ps[:])
            else:
                nc.scalar.activation(of[:], ps[:], IDN)
            row = qc * QCH + st * 128
            nc.sync.dma_start(out_d[row:row + 128, :], of[:])

        def attn_head(c, h, qtrc, nk, qoff=0, qw=QCH, key=None):
            """Pipelined attention for (chunk c, head h) over q columns
            [qoff, qoff+qw) of the chunk: scores run LAG k-tiles ahead of the
            PV matmuls; rowsums via bf16 pairs + f32 chain on vector; fires
            AG(key) at the end. Returns the rowsum matmul for ordering."""
            if key is None:
                key = c
            base = c * QCH + qoff      # global q start of this sub-range
            pv = ps_att.tile([128, QCH], f32, tag="att", name=f"pv{h}_{key}")
            pts = {}
            c0s = {}
            sacc = None      # running f32 sum of pair tiles
            pair_hold = None
            quad = None

            def emit_pv(j):
                c0 = c0s[j]
                nc.tensor.matmul(
                    pv[:, c0:qw], v_sb[j][:, h * 128:(h + 1) * 128],
                    pts[j][:, c0:qw], start=(j == 0), stop=(j == nk - 1))

            for ki in range(nk):
                rel = 128 * ki - base if causal else -1
                c0 = max(0, rel)
                c0s[ki] = c0
                ss = ps_s.tile([128, QCH], f32, tag="s",
                               name=f"ss{h}_{key}_{ki}")
                nc.tensor.matmul(
                    ss[:, c0:qw], ktr[h][:, ki * 128:(ki + 1) * 128],
                    qtrc[h][:, qoff + c0:qoff + qw], start=True, stop=True)
                if causal:
                    if rel >= 0:
                        nc.vector.tensor_add(ss[:, c0:c0 + 128],
                                             ss[:, c0:c0 + 128], tri_sb[:])
                else:
                    mt = ptp.tile([128, QCH], bf16, tag="mt",
                                  name=f"mt{h}_{key}_{ki}", bufs=4)
                    nc.sync.dma_start(
                        mt[:], mT_d[ki * 128:(ki + 1) * 128,
                                    c * QCH:(c + 1) * QCH])
                    nc.vector.tensor_add(ss[:], ss[:], mt[:])
                pt = ptp.tile([128, QCH], bf16, tag="pt",
                              name=f"pt{h}_{key}_{ki}")
                nc.scalar.activation(pt[:, c0:qw], ss[:, c0:qw], EXP)
                pts[ki] = pt
                # ---- rowsum accumulation (vector) ----
                if causal and rel >= 0:
                    # diagonal tiles fold into one bf16 tile
                    if c0 == 0:
                        quad = accp.tile([128, QCH], bf16, tag="quad",
                                         name=f"qd{h}_{key}")
                        nc.vector.tensor_copy(quad[:, 0:qw], pt[:, 0:qw])
                    else:
                        nc.vector.tensor_add(quad[:, c0:qw], quad[:, c0:qw],
                                             pt[:, c0:qw])
                else:
                    # full tiles pair up in bf16, then a f32 chain
                    if pair_hold is None:
                        pair_hold = pt
                    else:
                        pair = accp.tile([128, QCH], bf16, tag="pair",
                                         name=f"pr{h}_{key}_{ki}")
                        nc.vector.tensor_add(pair[:, 0:qw], pair_hold[:, 0:qw],
                                             pt[:, 0:qw])
                        pair_hold = None
                        if sacc is None:
                            sacc = accp.tile([128, QCH], f32, tag="sacc",
                                             name=f"sa{h}_{key}")
                            nc.vector.tensor_copy(sacc[:, 0:qw], pair[:, 0:qw])
                        else:
                            nc.vector.tensor_add(sacc[:, 0:qw], sacc[:, 0:qw],
                                                 pair[:, 0:qw])
                if ki >= LAG:
                    emit_pv(ki - LAG)
            for j in range(max(0, nk - LAG), nk):
                emit_pv(j)
            # odd leftover full tile (possible for sub-ranges)
            if pair_hold is not None:
                if sacc is None:
                    sacc = accp.tile([128, QCH], f32, tag="sacc",
                                     name=f"sa{h}_{key}")
                    nc.vector.tensor_copy(sacc[:, 0:qw], pair_hold[:, 0:qw])
                else:
                    nc.vector.tensor_add(sacc[:, 0:qw], sacc[:, 0:qw],
                                         pair_hold[:, 0:qw])
            # merge rowsums -> bf16 saccb for the ones-matmul reduce
            if causal and sacc is None:
                saccb = quad
            else:
                saccb = accp.tile([128, QCH], bf16, tag="saccb",
                                  name=f"sb{h}_{key}")
                if causal:
                    nc.vector.tensor_add(saccb[:, 0:qw], sacc[:, 0:qw],
                                         quad[:, 0:qw])
                else:
                    nc.vector.tensor_copy(saccb[:, 0:qw], sacc[:, 0:qw])
            # partition-reduce+broadcast the rowsums in one bf16 matmul
            sm = ps_att.tile([128, QCH], f32, tag="att", name=f"sm{h}_{key}")
            sm_mm = nc.tensor.matmul(sm[:, 0:qw], ones_sb[:], saccb[:, 0:qw],
                                     start=True, stop=True)
            recb = op.tile([128, QCH], f32, tag="recb",
                           name=f"recb{h}_{key}", bufs=2)
            nc.vector.reciprocal_approx_fast(out=recb[:, 0:qw], in_=sm[:, 0:qw])
            ot = op.tile([128, QCH], bf16, tag="ot", name=f"ot{h}_{key}",
                         bufs=3)
            nc.vector.tensor_mul(ot[:, 0:qw], pv[:, 0:qw], recb[:, 0:qw])
            gs = AG_GS.get(key, 2)
            pr, sl = divmod(h, gs)
            if sl == 0:
                agin = dram.tile([gs * 128, qw], bf16, tag=f"agin{key}_{pr}",
                                 name=f"agin{key}_{pr}")
                agin_pend[(key, pr)] = agin
            else:
                agin = agin_pend[(key, pr)]
            nc.sync.dma_start(agin[sl * 128:sl * 128 + 128, :], ot[:, 0:qw])
            if sl == gs - 1:
                ago = dram.tile([GPC * gs * 128, qw], bf16,
                                tag=f"agout{key}_{pr}",
                                name=f"agout{key}_{pr}")
                nc.gpsimd.collective_compute(
                    "AllGather", mybir.AluOpType.bypass,
                    replica_groups=REPLICA_GROUPS,
                    ins=[agin[:].opt()], outs=[ago[:].opt()])
                agout[(key, pr)] = ago
            return sm_mm

        # ---- main pipeline over q-chunks ----
        # Per block: V proj, Q proj, then per head: K proj for that head,
        # attention(c, h), outproj(c-1) st-groups. Interleaving K per head
        # keeps each engine's FIFO aligned with the intended overlap (exps
        # are never queued behind later proj copies).
        # Chunk 0's four AGs serialize on the CC core right after the tiny
        # attention(0), so its outproj groups wait until attention(1)'s later
        # heads to avoid stalling on the gather outputs.
        hTcs = {0: hTc0}
        if causal:
            for c in range(NQC):
                hTc, qtrc = proj_vq(c, hTcs)
                for h in range(HPC):
                    proj_head(c, hTc, wk_sb, bk_sb if use_bias else None,
                              ck_sb, sk_sb, ktr[h],
                              slice(c * QCH, (c + 1) * QCH), "k", h)
                # the CC core spends its first ~100us on runtime init, so
                # chunk 0/1's gathers land late; both chunks' outproj groups
                # run interleaved into attention(2) instead
                if c == 2:
                    load_xt(0)
                elif c == 3:
                    load_xt(2)
                sched = {1: [[], [], [], []],
                         2: [[(0, 0), (0, 1)], [(0, 2), (0, 3)],
                             [(1, 0), (1, 1)], [(1, 2), (1, 3)]]}.get(
                    c, [[(c - 1, 0)], [(c - 1, 1)], [(c - 1, 2)],
                        [(c - 1, 3)]])
                if c < NQC - 1:
                    for h in range(HPC):
                        attn_head(c, h, qtrc, 4 * c + 4)
                        if c > 0:
                            for qs, st in sched[h]:
                                outproj_group(qs, st, None)
                        if c == 2 and h == 1:
                            # xt(0) buffers free once outproj(0) is done
                            load_xt(1)
                else:
                    # last chunk: a 384-wide piece then a 128-wide piece, so
                    # most of the final outproj overlaps the second piece and
                    # the true tail is one small AllGather + one st-group
                    WA, WB = 384, 128
                    for h in range(HPC):
                        attn_head(c, h, qtrc, 4 * c + 3, qoff=0,
                                  qw=WA, key="La")
                        outproj_group(c - 1, h, None)
                    for h in range(HPC):
                        attn_head(c, h, qtrc, 4 * c + 4, qoff=WA,
                                  qw=WB, key="Lb")
                    load_xt("La", WA)
                    for st in range(3):
                        outproj_group(c, st, None, xt_key="La",
                                      stoff=st * 128, of_vec=True)
                    load_xt("Lb", WB)
                    outproj_group(c, 3, None, xt_key="Lb", stoff=0,
                                  of_vec=True)
        else:
            qtrcs = []
            for c in range(NQC):
                hTc, qtrc = proj_vq(c, hTcs)
                for h in range(HPC):
                    proj_head(c, hTc, wk_sb, bk_sb if use_bias else None,
                              ck_sb, sk_sb, ktr[h],
                              slice(c * QCH, (c + 1) * QCH), "k", h)
                qtrcs.append(qtrc)
            for c in range(NQC):
                if c > 0:
                    load_xt(c - 1)
                for h in range(HPC):
                    sm_mm = attn_head(c, h, qtrcs[c], NST)
                    if c > 0:
                        outproj_group(c - 1, h, sm_mm)
            load_xt(NQC - 1)
            for st in range(4):
                outproj_group(NQC - 1, st)

    nc.compile()
    return nc


def _get_built(causal: bool, use_bias: bool):
    key = (causal, use_bias)
    if key not in _built:
        _built[key] = _build(causal, use_bias)
    return _built[key]


def _prep_inputs(inputs, causal, use_bias):
    hs = np.asarray(inputs["hidden_states"], np.float32)
    fc = np.asarray(inputs["freqs_cis"], np.float32)
    Wq = np.asarray(inputs["Wq"], np.float32)
    Wk = np.asarray(inputs["Wk"], np.float32)
    Wv = np.asarray(inputs["Wv"], np.float32)
    Wo = np.asarray(inputs["Wo"], np.float32)
    bq = np.asarray(inputs["bq"], np.float32)
    bk = np.asarray(inputs["bk"], np.float32)
    bv = np.asarray(inputs["bv"], np.float32)
    bo = np.asarray(inputs["bo"], np.float32)

    # de-interleave permutation per 128-row head block: [0,2,..,126, 1,3,..,127]
    perm1 = np.concatenate([np.arange(0, DH, 2), np.arange(1, DH, 2)])
    permC = (np.arange(CW) // DH) * DH  # head base offsets
    perm = permC + perm1[np.arange(CW) % DH]

    scale = 1.0 / math.sqrt(DH)
    cos = np.concatenate([fc[:, :, 0].T, fc[:, :, 0].T])  # [128, S], dup halves
    sin = np.concatenate([fc[:, :, 1].T, fc[:, :, 1].T])
    cq = np.ascontiguousarray(cos * scale).astype(BF)
    sq = np.ascontiguousarray(sin * scale).astype(BF)
    ck = np.ascontiguousarray(cos).astype(BF)
    sk = np.ascontiguousarray(sin).astype(BF)

    if causal:
        tri = np.where(np.arange(128)[:, None] > np.arange(128)[None, :],
                       np.float32(NEG), np.float32(0.0)).astype(np.float32)
    else:
        maskT = np.ascontiguousarray(
            np.asarray(inputs["mask"], np.float32)[0, 0].T).astype(BF)

    hTb = [np.ascontiguousarray(hs[b].T).astype(BF) for b in range(B)]

    in_maps = []
    for c in range(NCORES):
        b, hg = divmod(c, GPC)
        sl = slice(CW * hg, CW * (hg + 1))
        wq_s = Wq[sl][perm]
        wk_s = Wk[sl][perm]
        m = {
            "hiddenT": hTb[b],
            "wqT": np.ascontiguousarray(wq_s.T).astype(BF),
            "wkT": np.ascontiguousarray(wk_s.T).astype(BF),
            "wvT": np.ascontiguousarray(Wv[sl].T).astype(BF),
            "woT": np.ascontiguousarray(Wo[sl].T).astype(BF),
            "cq": cq, "sq": sq, "ck": ck, "sk": sk,
        }
        if use_bias:
            m["bqp"] = np.ascontiguousarray(
                bq[sl][perm].reshape(HPC, 128).T).astype(np.float32)
            m["bkp"] = np.ascontiguousarray(
                bk[sl][perm].reshape(HPC, 128).T).astype(np.float32)
            m["bv2"] = bv[sl].reshape(1, CW).astype(np.float32)
            m["bo2"] = bo[sl].reshape(1, CW).astype(np.float32)
        if causal:
            m["dmask"] = tri
        else:
            m["maskT"] = maskT
        in_maps.append(m)
    return in_maps


def _is_causal(mask):
    mask = np.asarray(mask, np.float32)
    if mask.shape != (1, 1, S, S):
        return False
    m = mask[0, 0]
    expect = np.triu(np.full((S, S), np.float32(NEG)), k=1)
    return bool(np.array_equal(m, expect))


def run_on_cores(inputs, trace=False):
    """Compile+run; returns BassKernelResults."""
    from concourse.bass_utils import run_bass_kernel_spmd
    causal = _is_causal(inputs["mask"])
    use_bias = any(
        np.any(np.asarray(inputs[k])) for k in ("bq", "bk", "bv", "bo"))
    nc = _get_built(causal, use_bias)
    in_maps = _prep_inputs(inputs, causal, use_bias)
    r = run_bass_kernel_spmd(nc, in_maps, list(range(NCORES)), trace=trace)
    return r


def kernel(**inputs) -> np.ndarray:
    r = run_on_cores(inputs)
    out = np.empty((B, S, D), np.float32)
    for c in range(NCORES):
        b, hg = divmod(c, GPC)
        out[b, :, CW * hg:CW * (hg + 1)] = r.results[c]["out"]
    return out


# revision 44
# speedup vs baseline: 1.1069x; 1.0360x over previous
"""Self-contained Trainium2 Bass kernel: 16-head attention with RoPE (B=2, S=2048, D=2048).

Sharding: 8 cores = 2 (batch) x 4 (head groups of 4 heads / 512 cols).
Per core: QKV projections for its head group -> RoPE -> causal attention ->
per-(chunk, head) AllGather of attention outputs (X^T) within the 4-core
batch group -> column-sharded output projection. Host assembles by
concatenation only.

Pipeline (per q-chunk c of 512):
  proj V(c), Q(c)+RoPE, K(c)+RoPE, then per head: pipelined attention where
  scores run LAG k-tiles ahead of the PV matmuls so the PE queue never
  head-blocks on the exp chain; causal tri-mask adds and softmax rowsums
  (bf16 pair-adds + f32 chain) run on Vector; exp is the only Scalar work
  during attention. Attention outputs AllGather per head-pair (the CC core
  spends its first ~100us on runtime init, so chunk 0/1's outproj groups
  are deferred to attention(2)); outproj st-groups interleave into the next
  chunk's attention and the scheduler hoists their matmuls into exp-wait
  stalls. The last chunk runs as 384+128-wide q-pieces with per-head /
  single AllGathers so the true tail is one small gather + one st-group.
  A tiny warmup AllGather absorbs the CC cold-start under proj(0).

Dataflow is fully "transposed" so no on-chip transposes are needed:
  hiddenT [d, s] (host-pretransposed, bf16), streamed per chunk
  QT/KT   [dh, s] per head  (projection emits head-dim-major directly)
  S^T     [k, q] scores     (lhsT = KT tile, rhs = QT)
  P^T     [k, q] = exp(S^T + mask^T)   (no max subtraction; scores are O(1))
  colsums via all-ones [128,128] matmul -> sums arrive partition-broadcast
  O^T     [dh, q] = V^T @ P^T          (lhsT = V natural [s, dh])
  X^T     per-head AllGather on first axis
  out     [s, oc] (lhsT = X^T block, rhs = WoT)
RoPE de-interleave is folded into a host-side row permutation of Wq/Wk.
1/sqrt(DH) is folded into the Q rope tables.
"""

import math
from contextlib import ExitStack

import numpy as np
import ml_dtypes

B, S, D, H, DH = 2, 2048, 2048, 16, 128
NCORES = 8
GPC = 4            # cores per tensor-parallel group
HPC = H // GPC     # heads per core (4)
CW = HPC * DH      # 512 columns per core
NEG = -1e9
BF = ml_dtypes.bfloat16
QCH = 512          # q-chunk (moving free dim)
NQC = S // QCH     # 4
NDT = D // 128     # 16 d-tiles
NST = S // 128     # 16 s-tiles
LAG = 3            # PV matmul trails its score matmul by LAG k-tiles

REPLICA_GROUPS = [[0, 1, 2, 3], [4, 5, 6, 7]]

_built = {}


def _build(causal: bool, use_bias: bool):
    import concourse.bass as bass
    import concourse.tile as tile
    from concourse import bacc, mybir
    from concourse.tile_rust import add_dep_helper

    f32, bf16 = mybir.dt.float32, mybir.dt.bfloat16
    EXP = mybir.ActivationFunctionType.Exp
    IDN = mybir.ActivationFunctionType.Identity

    nc = bacc.Bacc("TRN2", target_bir_lowering=False, debug=False,
                   num_devices=NCORES)

    hT_d = nc.dram_tensor("hiddenT", [D, S], bf16, kind="ExternalInput")
    wq_d = nc.dram_tensor("wqT", [D, CW], bf16, kind="ExternalInput")
    wk_d = nc.dram_tensor("wkT", [D, CW], bf16, kind="ExternalInput")
    wv_d = nc.dram_tensor("wvT", [D, CW], bf16, kind="ExternalInput")
    wo_d = nc.dram_tensor("woT", [D, CW], bf16, kind="ExternalInput")
    cq_d = nc.dram_tensor("cq", [128, S], bf16, kind="ExternalInput")
    sq_d = nc.dram_tensor("sq", [128, S], bf16, kind="ExternalInput")
    ck_d = nc.dram_tensor("ck", [128, S], bf16, kind="ExternalInput")
    sk_d = nc.dram_tensor("sk", [128, S], bf16, kind="ExternalInput")
    if use_bias:
        bq_d = nc.dram_tensor("bqp", [128, HPC], f32, kind="ExternalInput")
        bk_d = nc.dram_tensor("bkp", [128, HPC], f32, kind="ExternalInput")
        bv_d = nc.dram_tensor("bv2", [1, CW], f32, kind="ExternalInput")
        bo_d = nc.dram_tensor("bo2", [1, CW], f32, kind="ExternalInput")
    if causal:
        dm_d = nc.dram_tensor("dmask", [128, 128], f32, kind="ExternalInput")
    else:
        mT_d = nc.dram_tensor("maskT", [S, S], bf16, kind="ExternalInput")
    out_d = nc.dram_tensor("out", [S, CW], f32, kind="ExternalOutput")

    with tile.TileContext(nc) as tc, ExitStack() as ctx:
        hp = ctx.enter_context(tc.tile_pool(name="hp", bufs=2 * NDT))
        xp = ctx.enter_context(tc.tile_pool(name="xp", bufs=NDT + 4))
        wp = ctx.enter_context(tc.tile_pool(name="wp", bufs=4 * NDT))
        qkp = ctx.enter_context(tc.tile_pool(name="qkp", bufs=2 * HPC))
        vp = ctx.enter_context(tc.tile_pool(name="vp", bufs=NST))
        cst = ctx.enter_context(tc.tile_pool(name="cst", bufs=1))
        ptp = ctx.enter_context(tc.tile_pool(name="ptp", bufs=7))
        accp = ctx.enter_context(tc.tile_pool(name="accp", bufs=2))
        rp = ctx.enter_context(tc.tile_pool(name="rp", bufs=2))
        op = ctx.enter_context(tc.tile_pool(name="op", bufs=3))
        ps_mm = ctx.enter_context(tc.tile_pool(name="ps_mm", bufs=2, space="PSUM"))
        ps_s = ctx.enter_context(tc.tile_pool(name="ps_s", bufs=3, space="PSUM"))
        ps_att = ctx.enter_context(tc.tile_pool(name="ps_att", bufs=3, space="PSUM"))
        dram = ctx.enter_context(tc.tile_pool(name="dram", bufs=1, space="DRAM"))

        # ---- first-needed data first: hT(chunk 0) + Wv interleaved ----
        wv_sb, wq_sb, wk_sb, wo_sb = [], [], [], []
        hTc0 = []
        for dt in range(NDT):
            w = wp.tile([128, CW], bf16, tag="w", name=f"wv{dt}")
            nc.sync.dma_start(w[:], wv_d[dt * 128:(dt + 1) * 128, :])
            wv_sb.append(w)
            t = hp.tile([128, QCH], bf16, tag="hT", name=f"hT0_{dt}")
            nc.sync.dma_start(t[:], hT_d[dt * 128:(dt + 1) * 128, 0:QCH])
            hTc0.append(t)
        # Q weights next (used before K)
        for dt in range(NDT):
            w = wp.tile([128, CW], bf16, tag="w", name=f"wq{dt}")
            nc.sync.dma_start(w[:], wq_d[dt * 128:(dt + 1) * 128, :])
            wq_sb.append(w)

        # ---- constants ----
        cq_sb = cst.tile([128, S], bf16, tag="cq", name="cq_sb")
        sq_sb = cst.tile([128, S], bf16, tag="sq", name="sq_sb")
        ck_sb = cst.tile([128, S], bf16, tag="ck", name="ck_sb")
        sk_sb = cst.tile([128, S], bf16, tag="sk", name="sk_sb")
        nc.sync.dma_start(cq_sb[:], cq_d[:])
        nc.sync.dma_start(sq_sb[:], sq_d[:])
        for dt in range(NDT):
            w = wp.tile([128, CW], bf16, tag="w", name=f"wk{dt}")
            nc.sync.dma_start(w[:], wk_d[dt * 128:(dt + 1) * 128, :])
            wk_sb.append(w)
        nc.sync.dma_start(ck_sb[:], ck_d[:])
        nc.sync.dma_start(sk_sb[:], sk_d[:])
        if use_bias:
            bq_sb = cst.tile([128, HPC], f32, tag="bq", name="bq_sb")
            bk_sb = cst.tile([128, HPC], f32, tag="bk", name="bk_sb")
            bv_sb = cst.tile([1, CW], f32, tag="bv", name="bv_sb")
            bo_sb = cst.tile([1, CW], f32, tag="bo", name="bo_sb")
            nc.sync.dma_start(bq_sb[:], bq_d[:])
            nc.sync.dma_start(bk_sb[:], bk_d[:])
            nc.sync.dma_start(bv_sb[:], bv_d[:])
            nc.sync.dma_start(bo_sb[:], bo_d[:])
            bvb_sb = cst.tile([128, CW], f32, tag="bvb", name="bvb_sb")
            bob_sb = cst.tile([128, CW], f32, tag="bob", name="bob_sb")
            nc.gpsimd.partition_broadcast(bvb_sb[:], bv_sb[0:1, :])
            nc.gpsimd.partition_broadcast(bob_sb[:], bo_sb[0:1, :])
        ones_sb = cst.tile([128, 128], bf16, tag="ones", name="ones_sb")
        nc.vector.memset(ones_sb[:], 1.0)
        # tiny warmup AllGather: pays the CC core's cold-start cost under
        # chunk 0's projections instead of before its first real gather
        wg_in = dram.tile([512, 64], bf16, tag="wg_in", name="wg_in")
        wg_out = dram.tile([GPC * 512, 64], bf16, tag="wg_out", name="wg_out")
        for r in range(4):
            nc.sync.dma_start(wg_in[r * 128:(r + 1) * 128, :],
                              ones_sb[:, 0:64])
        nc.gpsimd.collective_compute(
            "AllGather", mybir.AluOpType.bypass,
            replica_groups=REPLICA_GROUPS,
            ins=[wg_in[:].opt()], outs=[wg_out[:].opt()])
        if causal:
            tri_sb = cst.tile([128, 128], f32, tag="tri", name="tri_sb")
            nc.sync.dma_start(tri_sb[:], dm_d[:])

        # Wo streams in behind everything else
        for dt in range(NDT):
            t = wp.tile([128, CW], bf16, tag="w", name=f"wo{dt}")
            nc.sync.dma_start(t[:], wo_d[dt * 128:(dt + 1) * 128, :])
            wo_sb.append(t)

        # persistent KT (written chunk by chunk; all history needed) and V;
        # QT is per-chunk only
        ktr = [qkp.tile([128, S], bf16, tag="qk", name=f"ktr{m}", bufs=HPC)
               for m in range(HPC)]
        v_sb = [None] * NST

        def rope(dst, dsl, c_sb, s_sb, c, prefix, m):
            """RoPE dst[:, dsl] in place (4 DVE ops + 2 DMA swaps).
            rows 0:64 = "real"(a), 64:128 = "imag"(b):
              a' = a*cos - b*sin ; b' = b*cos + a*sin"""
            csl = slice(c * QCH, (c + 1) * QCH)
            t1 = rp.tile([128, QCH], bf16, tag="t1", name=f"{prefix}t1{m}_{c}")
            tc_ = rp.tile([128, QCH], bf16, tag="tc", name=f"{prefix}tc{m}_{c}")
            ts_ = rp.tile([128, QCH], bf16, tag="ts", name=f"{prefix}ts{m}_{c}")
            nc.sync.dma_start(t1[0:64, :], dst[64:128, dsl])
            nc.sync.dma_start(t1[64:128, :], dst[0:64, dsl])
            nc.vector.tensor_mul(tc_[:], dst[:, dsl], c_sb[:, csl])  # [a*c;b*c]
            nc.vector.tensor_mul(ts_[:], t1[:], s_sb[:, csl])        # [b*s;a*s]
            nc.vector.tensor_sub(dst[0:64, dsl], tc_[0:64, :], ts_[0:64, :])
            nc.vector.tensor_add(dst[64:128, dsl], tc_[64:128, :], ts_[64:128, :])

        def proj_head(c, hTc, w_sb, b_sb, c_sb, s_sb, dst, dsl, prefix, m):
            """One head's Q or K projection (16 accumulating MMs) + rope."""
            ps = ps_mm.tile([128, QCH], f32, tag="mm", name=f"{prefix}ps{m}_{c}")
            for dt in range(NDT):
                nc.tensor.matmul(ps[:], w_sb[dt][:, m * 128:(m + 1) * 128],
                                 hTc[dt][:], start=(dt == 0), stop=(dt == NDT - 1))
            if use_bias:
                nc.scalar.activation(dst[:, dsl], ps[:], IDN, bias=b_sb[:, m:m + 1])
            else:
                nc.scalar.activation(dst[:, dsl], ps[:], IDN)
            rope(dst, dsl, c_sb, s_sb, c, prefix, m)

        def proj_vq(c, hTcs):
            """Prefetch hT(c+1); project V (4 s-tiles) and all Q heads."""
            if c + 1 < NQC:
                nsl = slice((c + 1) * QCH, (c + 2) * QCH)
                nxt = []
                for dt in range(NDT):
                    t = hp.tile([128, QCH], bf16, tag="hT",
                                name=f"hT{c + 1}_{dt}")
                    nc.sync.dma_start(t[:], hT_d[dt * 128:(dt + 1) * 128, nsl])
                    nxt.append(t)
                hTcs[c + 1] = nxt
            hTc = hTcs[c]
            for sti in range(4):
                st = 4 * c + sti
                ps = ps_mm.tile([128, CW], f32, tag="mm", name=f"psv{st}")
                for dt in range(NDT):
                    nc.tensor.matmul(ps[:], hTc[dt][:, sti * 128:(sti + 1) * 128],
                                     wv_sb[dt][:],
                                     start=(dt == 0), stop=(dt == NDT - 1))
                vt = vp.tile([128, CW], bf16, tag="v", name=f"v{st}")
                if use_bias:
                    nc.vector.tensor_add(vt[:], ps[:], bvb_sb[:])
                else:
                    nc.vector.tensor_copy(vt[:], ps[:])
                v_sb[st] = vt
            qtrc = [qkp.tile([128, QCH], bf16, tag="qtc", name=f"qtc{c}_{m}",
                             bufs=(HPC + 1 if causal else 4 * HPC))
                    for m in range(HPC)]
            for m in range(HPC):
                proj_head(c, hTc, wq_sb, bq_sb if use_bias else None,
                          cq_sb, sq_sb, qtrc[m], slice(0, QCH), "q", m)
            return hTc, qtrc

        # per-(chunk, head-pair) AllGather buffers
        agout = {}     # (key, group) -> DRAM tile [GPC*gs*128, qw]
        agin_pend = {}
        AG_GS = {2: 1, "La": 1, "Lb": 1}

        xt_tiles = {}

        def load_xt(key, qw=QCH):
            """Fetch the gathered X^T tiles for AG key (one contiguous DMA
            per (head, rank) block); shared by the matching outproj groups."""
            for hh in range(HPC):
                for g in range(GPC):
                    t = xp.tile([128, QCH], bf16, tag="xt",
                                name=f"xt{key}_{hh}_{g}")
                    gs = AG_GS.get(key, 2)
                    row = g * gs * 128 + (hh % gs) * 128
                    nc.sync.dma_start(
                        t[:, 0:qw], agout[(key, hh // gs)][row:row + 128, :])
                    xt_tiles[(key, hh, g)] = t

        def outproj_group(qc, st, after_mm=None, xt_key=None, stoff=None,
                          of_vec=False):
            """out rows [qc*QCH + st*128 : +128] = X^T chunk-slice @ WoT.
            dt accumulation ordered h-major so late AGs are needed last."""
            if xt_key is None:
                xt_key = qc
            if stoff is None:
                stoff = st * 128
            stsl = slice(stoff, stoff + 128)
            ps = ps_mm.tile([128, CW], f32, tag="mm", name=f"pso{qc}_{st}")
            n = 0
            for hh in range(HPC):
                for g in range(GPC):
                    dt = 4 * g + hh
                    mm = nc.tensor.matmul(ps[:], xt_tiles[(xt_key, hh, g)][:, stsl],
                                          wo_sb[dt][:],
                                          start=(n == 0), stop=(n == NDT - 1))
                    if n == 0 and after_mm is not None:
                        # keep outproj behind the current attention head in PE
                        # program order; the static scheduler would hoist it
                        # ahead of the gather outputs otherwise
                        add_dep_helper(mm.ins, after_mm.ins, sync=False,
                                       reason="outproj after attn head")
                    n += 1
            of = op.tile([128, CW], f32, tag="of", name=f"of{qc}_{st}", bufs=2)
            if use_bias:
                nc.vector.tensor_add(of[:], ps[:], bob_sb[:])
            elif of_vec:
                nc.vector.tensor_copy(of[:], ps[:])
            else:
                nc.scalar.activation(of[:], ps[:], IDN)
            row = qc * QCH + st * 128
            nc.sync.dma_start(out_d[row:row + 128, :], of[:])

        def attn_head(c, h, qtrc, nk, qoff=0, qw=QCH, key=None):
            """Pipelined attention for (chunk c, head h) over q columns
            [qoff, qoff+qw) of the chunk: scores run LAG k-tiles ahead of the
            PV matmuls; rowsums via bf16 pairs + f32 chain on vector; fires
            AG(key) at the end. Returns the rowsum matmul for ordering."""
            if key is None:
                key = c
            base = c * QCH + qoff      # global q start of this sub-range
            pv = ps_att.tile([128, QCH], f32, tag="att", name=f"pv{h}_{key}")
            pts = {}
            c0s = {}
            sacc = None      # running f32 sum of pair tiles
            pair_hold = None
            quad = None

            def emit_pv(j):
                c0 = c0s[j]
                nc.tensor.matmul(
                    pv[:, c0:qw], v_sb[j][:, h * 128:(h + 1) * 128],
                    pts[j][:, c0:qw], start=(j == 0), stop=(j == nk - 1))

            for ki in range(nk):
                rel = 128 * ki - base if causal else -1
                c0 = max(0, rel)
                c0s[ki] = c0
                ss = ps_s.tile([128, QCH], f32, tag="s",
                               name=f"ss{h}_{key}_{ki}")
                nc.tensor.matmul(
                    ss[:, c0:qw], ktr[h][:, ki * 128:(ki + 1) * 128],
                    qtrc[h][:, qoff + c0:qoff + qw], start=True, stop=True)
                if causal:
                    if rel >= 0:
                        nc.vector.tensor_add(ss[:, c0:c0 + 128],
                                             ss[:, c0:c0 + 128], tri_sb[:])
                else:
                    mt = ptp.tile([128, QCH], bf16, tag="mt",
                                  name=f"mt{h}_{key}_{ki}", bufs=4)
                    nc.sync.dma_start(
                        mt[:], mT_d[ki * 128:(ki + 1) * 128,
                                    c * QCH:(c + 1) * QCH])
                    nc.vector.tensor_add(ss[:], ss[:], mt[:])
                pt = ptp.tile([128, QCH], bf16, tag="pt",
                              name=f"pt{h}_{key}_{ki}")
                nc.scalar.activation(pt[:, c0:qw], ss[:, c0:qw], EXP)
                pts[ki] = pt
                # ---- rowsum accumulation (vector) ----
                if causal and rel >= 0:
                    # diagonal tiles fold into one bf16 tile
                    if c0 == 0:
                        quad = accp.tile([128, QCH], bf16, tag="quad",
                                         name=f"qd{h}_{key}")
                        nc.vector.tensor_copy(quad[:, 0:qw], pt[:, 0:qw])
                    else:
                        nc.vector.tensor_add(quad[:, c0:qw], quad[:, c0:qw],
                                             pt[:, c0:qw])
                else:
                    # full tiles pair up in bf16, then a f32 chain
                    if pair_hold is None:
                        pair_hold = pt
                    else:
                        pair = accp.tile([128, QCH], bf16, tag="pair",
                                         name=f"pr{h}_{key}_{ki}")
                        nc.vector.tensor_add(pair[:, 0:qw], pair_hold[:, 0:qw],
                                             pt[:, 0:qw])
                        pair_hold = None
                        if sacc is None:
                            sacc = accp.tile([128, QCH], f32, tag="sacc",
                                             name=f"sa{h}_{key}")
                            nc.vector.tensor_copy(sacc[:, 0:qw], pair[:, 0:qw])
                        else:
                            nc.vector.tensor_add(sacc[:, 0:qw], sacc[:, 0:qw],
                                                 pair[:, 0:qw])
                if ki >= LAG:
                    emit_pv(ki - LAG)
            for j in range(max(0, nk - LAG), nk):
                emit_pv(j)
            # odd leftover full tile (possible for sub-ranges)
            if pair_hold is not None:
                if sacc is None:
                    sacc = accp.tile([128, QCH], f32, tag="sacc",
                                     name=f"sa{h}_{key}")
                    nc.vector.tensor_copy(sacc[:, 0:qw], pair_hold[:, 0:qw])
                else:
                    nc.vector.tensor_add(sacc[:, 0:qw], sacc[:, 0:qw],
                                         pair_hold[:, 0:qw])
            # merge rowsums -> bf16 saccb for the ones-matmul reduce
            if causal and sacc is None:
                saccb = quad
            else:
                saccb = accp.tile([128, QCH], bf16, tag="saccb",
                                  name=f"sb{h}_{key}")
                if causal:
                    nc.vector.tensor_add(saccb[:, 0:qw], sacc[:, 0:qw],
                                         quad[:, 0:qw])
                else:
                    nc.vector.tensor_copy(saccb[:, 0:qw], sacc[:, 0:qw])
            # partition-reduce+broadcast the rowsums in one bf16 matmul
            sm = ps_att.tile([128, QCH], f32, tag="att", name=f"sm{h}_{key}")
            sm_mm = nc.tensor.matmul(sm[:, 0:qw], ones_sb[:], saccb[:, 0:qw],
                                     start=True, stop=True)
            recb = op.tile([128, QCH], f32, tag="recb",
                           name=f"recb{h}_{key}", bufs=2)
            nc.vector.reciprocal_approx_fast(out=recb[:, 0:qw], in_=sm[:, 0:qw])
            ot = op.tile([128, QCH], bf16, tag="ot", name=f"ot{h}_{key}",
                         bufs=3)
            nc.vector.tensor_mul(ot[:, 0:qw], pv[:, 0:qw], recb[:, 0:qw])
            gs = AG_GS.get(key, 2)
            pr, sl = divmod(h, gs)
            if sl == 0:
                agin = dram.tile([gs * 128, qw], bf16, tag=f"agin{key}_{pr}",
                                 name=f"agin{key}_{pr}")
                agin_pend[(key, pr)] = agin
            else:
                agin = agin_pend[(key, pr)]
            nc.sync.dma_start(agin[sl * 128:sl * 128 + 128, :], ot[:, 0:qw])
            if sl == gs - 1:
                ago = dram.tile([GPC * gs * 128, qw], bf16,
                                tag=f"agout{key}_{pr}",
                                name=f"agout{key}_{pr}")
                nc.gpsimd.collective_compute(
                    "AllGather", mybir.AluOpType.bypass,
                    replica_groups=REPLICA_GROUPS,
                    ins=[agin[:].opt()], outs=[ago[:].opt()])
                agout[(key, pr)] = ago
            return sm_mm

        # ---- main pipeline over q-chunks ----
        # Per block: V proj, Q proj, then per head: K proj for that head,
        # attention(c, h), outproj(c-1) st-groups. Interleaving K per head
        # keeps each engine's FIFO aligned with the intended overlap (exps
        # are never queued behind later proj copies).
        # Chunk 0's four AGs serialize on the CC core right after the tiny
        # attention(0), so its outproj groups wait until attention(1)'s later
        # heads to avoid stalling on the gather outputs.
        hTcs = {0: hTc0}
        if causal:
            for c in range(NQC):
                hTc, qtrc = proj_vq(c, hTcs)
                for h in range(HPC):
                    proj_head(c, hTc, wk_sb, bk_sb if use_bias else None,
                              ck_sb, sk_sb, ktr[h],
                              slice(c * QCH, (c + 1) * QCH), "k", h)
                # the CC core spends its first ~100us on runtime init, so
                # chunk 0/1's gathers land late; both chunks' outproj groups
                # run interleaved into attention(2) instead
                if c == 2:
                    load_xt(0)
                elif c == 3:
                    load_xt(2)
                sched = {1: [[], [], [], []],
                         2: [[(0, 0), (0, 1)], [(0, 2), (0, 3)],
                             [(1, 0), (1, 1)], [(1, 2), (1, 3)]]}.get(
                    c, [[(c - 1, 0)], [(c - 1, 1)], [(c - 1, 2)],
                        [(c - 1, 3)]])
                if c < NQC - 1:
                    for h in range(HPC):
                        attn_head(c, h, qtrc, 4 * c + 4)
                        if c > 0:
                            for qs, st in sched[h]:
                                outproj_group(qs, st, None)
                        if c == 2 and h == 1:
                            # xt(0) buffers free once outproj(0) is done
                            load_xt(1)
                else:
                    # last chunk: a 384-wide piece then a 128-wide piece, so
                    # most of the final outproj overlaps the second piece and
                    # the true tail is one small AllGather + one st-group
                    WA, WB = 384, 128
                    for h in range(HPC):
                        attn_head(c, h, qtrc, 4 * c + 3, qoff=0,
                                  qw=WA, key="La")
                        outproj_group(c - 1, h, None)
                    for h in range(HPC):
                        attn_head(c, h, qtrc, 4 * c + 4, qoff=WA,
                                  qw=WB, key="Lb")
                    load_xt("La", WA)
                    for st in range(3):
                        outproj_group(c, st, None, xt_key="La",
                                      stoff=st * 128, of_vec=True)
                    load_xt("Lb", WB)
                    outproj_group(c, 3, None, xt_key="Lb", stoff=0,
                                  of_vec=True)
        else:
            qtrcs = []
            for c in range(NQC):
                hTc, qtrc = proj_vq(c, hTcs)
                for h in range(HPC):
                    proj_head(c, hTc, wk_sb, bk_sb if use_bias else None,
                              ck_sb, sk_sb, ktr[h],
                              slice(c * QCH, (c + 1) * QCH), "k", h)
                qtrcs.append(qtrc)
            for c in range(NQC):
                if c > 0:
                    load_xt(c - 1)
                for h in range(HPC):
                    sm_mm = attn_head(c, h, qtrcs[c], NST)
                    if c > 0:
                        outproj_group(c - 1, h, sm_mm)
            load_xt(NQC - 1)
            for st in range(4):
                outproj_group(NQC - 1, st)

    nc.compile()
    return nc


def _get_built(causal: bool, use_bias: bool):
    key = (causal, use_bias)
    if key not in _built:
        _built[key] = _build(causal, use_bias)
    return _built[key]


def _prep_inputs(inputs, causal, use_bias):
    hs = np.asarray(inputs["hidden_states"], np.float32)
    fc = np.asarray(inputs["freqs_cis"], np.float32)
    Wq = np.asarray(inputs["Wq"], np.float32)
    Wk = np.asarray(inputs["Wk"], np.float32)
    Wv = np.asarray(inputs["Wv"], np.float32)
    Wo = np.asarray(inputs["Wo"], np.float32)
    bq = np.asarray(inputs["bq"], np.float32)
    bk = np.asarray(inputs["bk"], np.float32)
    bv = np.asarray(inputs["bv"], np.float32)
    bo = np.asarray(inputs["bo"], np.float32)

    # de-interleave permutation per 128-row head block: [0,2,..,126, 1,3,..,127]
    perm1 = np.concatenate([np.arange(0, DH, 2), np.arange(1, DH, 2)])
    permC = (np.arange(CW) // DH) * DH  # head base offsets
    perm = permC + perm1[np.arange(CW) % DH]

    scale = 1.0 / math.sqrt(DH)
    cos = np.concatenate([fc[:, :, 0].T, fc[:, :, 0].T])  # [128, S], dup halves
    sin = np.concatenate([fc[:, :, 1].T, fc[:, :, 1].T])
    cq = np.ascontiguousarray(cos * scale).astype(BF)
    sq = np.ascontiguousarray(sin * scale).astype(BF)
    ck = np.ascontiguousarray(cos).astype(BF)
    sk = np.ascontiguousarray(sin).astype(BF)

    if causal:
        tri = np.where(np.arange(128)[:, None] > np.arange(128)[None, :],
                       np.float32(NEG), np.float32(0.0)).astype(np.float32)
    else:
        maskT = np.ascontiguousarray(
            np.asarray(inputs["mask"], np.float32)[0, 0].T).astype(BF)

    hTb = [np.ascontiguousarray(hs[b].T).astype(BF) for b in range(B)]

    in_maps = []
    for c in range(NCORES):
        b, hg = divmod(c, GPC)
        sl = slice(CW * hg, CW * (hg + 1))
        wq_s = Wq[sl][perm]
        wk_s = Wk[sl][perm]
        m = {
            "hiddenT": hTb[b],
            "wqT": np.ascontiguousarray(wq_s.T).astype(BF),
            "wkT": np.ascontiguousarray(wk_s.T).astype(BF),
            "wvT": np.ascontiguousarray(Wv[sl].T).astype(BF),
            "woT": np.ascontiguousarray(Wo[sl].T).astype(BF),
            "cq": cq, "sq": sq, "ck": ck, "sk": sk,
        }
        if use_bias:
            m["bqp"] = np.ascontiguousarray(
                bq[sl][perm].reshape(HPC, 128).T).astype(np.float32)
            m["bkp"] = np.ascontiguousarray(
                bk[sl][perm].reshape(HPC, 128).T).astype(np.float32)
            m["bv2"] = bv[sl].reshape(1, CW).astype(np.float32)
            m["bo2"] = bo[sl].reshape(1, CW).astype(np.float32)
        if causal:
            m["dmask"] = tri
        else:
            m["maskT"] = maskT
        in_maps.append(m)
    return in_maps


def _is_causal(mask):
    mask = np.asarray(mask, np.float32)
    if mask.shape != (1, 1, S, S):
        return False
    m = mask[0, 0]
    expect = np.triu(np.full((S, S), np.float32(NEG)), k=1)
    return bool(np.array_equal(m, expect))


def run_on_cores(inputs, trace=False):
    """Compile+run; returns BassKernelResults."""
    from concourse.bass_utils import run_bass_kernel_spmd
    causal = _is_causal(inputs["mask"])
    use_bias = any(
        np.any(np.asarray(inputs[k])) for k in ("bq", "bk", "bv", "bo"))
    nc = _get_built(causal, use_bias)
    in_maps = _prep_inputs(inputs, causal, use_bias)
    r = run_bass_kernel_spmd(nc, in_maps, list(range(NCORES)), trace=trace)
    return r


def kernel(**inputs) -> np.ndarray:
    r = run_on_cores(inputs)
    out = np.empty((B, S, D), np.float32)
    for c in range(NCORES):
        b, hg = divmod(c, GPC)
        out[b, :, CW * hg:CW * (hg + 1)] = r.results[c]["out"]
    return out
